# revision 2
# baseline (speedup 1.0000x reference)
"""GaussianMixture log-likelihood kernel for 8 TRN2 NeuronCores, v2.

Math (per point x, cluster k):
  d_ik = -0.5 ||L_k^T x - b_k||^2,  b_k = L_k^T c_k,  L = cov_inv_sqrt
  ll_i = log sum_k coef_k exp(d_ik) - thr,  coef_k = pr_k |det L_k|

Device strategy (data-parallel over N, 8192 points/core), "orientation B":
  - X^T lives in SBUF as [128, 4096] bf16: rows 0:64 = features of the first
    512 points of each 1024-point column-chunk, rows 64:128 = the second 512.
  - Whitening matmuls run as row-packed pairs (tile_position rows 0/64): two
    K=64 matmuls concurrently, producing Z^T chunks [128 = 2 clusters x 64
    coords, 512 points] in PSUM. G is pre-scaled by sqrt(0.5).
  - Squares: ACT activation(Square, bias=-sqrt(.5) b per partition) or DVE
    (tensor_scalar_add bias then tensor_tensor mult) -> 0.5(z-b)^2 bf16.
  - c-reduction on the PE: selector matmuls (SEL = -1 block pattern)
    accumulate -0.5||z-b||^2 into a [128 = 4 groups x 32 clusters, 512] PSUM
    slab; the two per-slot selector mms use different col-groups and run
    concurrently.
  - Epilogue per slab, all ACT: E = exp(d0 + (ln coef + 84)) bf16, s = ones
    matmul over the 32-cluster groups, ll = ln(s) - 84 - thr, DMA out.
"""

import sys

sys.path.insert(0, "/opt/trn_rl_repo")

import numpy as np

from concourse import bacc, mybir
from concourse.tile import TileContext
from concourse.bass_utils import run_bass_kernel_spmd

N, D, K = 65536, 64, 32
NCORES = 8
NLOC = N // NCORES            # 8192 points per core
NCH = 8                       # column chunks of 512 XT2 cols (1024 points)
NQ = 16                       # cluster chunk-pairs (2 clusters x 64 coords)
NSLAB = 4                     # slabs of 2 col-chunks (4 point-groups of 512)
EXPB = 38.0

F32 = mybir.dt.float32
BF16 = mybir.dt.bfloat16

# square-path assignment per tile index mod 16:
#   A = ACT square(bias), D = DVE ts_add + DVE tt mult, P = DVE ts_add + Pool tt
_PATH16 = ['A', 'P', 'A', 'D', 'A', 'P', 'A', 'A',
           'P', 'A', 'D', 'A', 'P', 'A', 'A', 'P']


def _tile_path(c: int, q: int) -> str:
    return _PATH16[(c * NQ + q) % 16]


def _build_nc(threshold_f: float):
    nc = bacc.Bacc()

    xa_d = nc.declare_dram_parameter("xa", [128, 4096], BF16, isOutput=False)
    # gsb: GG [128,2048] | SEL [128,512] | ones4 [128,4] | ZZ [128,128]
    gsb_d = nc.declare_dram_parameter("gsb", [128, 2048 + 512 + 4 + 128], BF16, isOutput=False)
    # fbias: bbias [128,16] | ebias [128,1]
    fb_d = nc.declare_dram_parameter("fb", [128, 17], F32, isOutput=False)
    out_d = nc.declare_dram_parameter("out", [16, 512], F32, isOutput=True)

    with TileContext(nc) as tc:
        with (
            tc.tile_pool(name="const", bufs=1) as cpool,
            tc.tile_pool(name="z2", bufs=9) as z2pool,
            tc.tile_pool(name="zs", bufs=6) as zspool,
            tc.tile_pool(name="et", bufs=2) as etpool,
            tc.tile_pool(name="fin", bufs=2) as finpool,
            tc.tile_pool(name="psz", bufs=3, space="PSUM") as zpool,
            tc.tile_pool(name="psd", bufs=1, space="PSUM") as dpool,
            tc.tile_pool(name="pss", bufs=1, space="PSUM") as spool,
        ):
            xt2 = cpool.tile([128, 4096], BF16)
            nc.sync.dma_start(out=xt2[:, :], in_=xa_d[:, :])
            gsb = cpool.tile([128, 2048 + 512 + 4 + 128], BF16)
            nc.sync.dma_start(out=gsb[:, :], in_=gsb_d[:, :])
            fb = cpool.tile([128, 17], F32)
            nc.sync.dma_start(out=fb[:, :], in_=fb_d[:, :])

            GG = gsb[:, 0:2048]
            SEL = gsb[:, 2048:2560]
            ones4 = gsb[:, 2560:2564]
            ZZ = gsb[:, 2564:2692]
            bbias = fb[:, 0:16]
            ebias = fb[:, 16:17]
            st16 = cpool.tile([128, 512], F32)  # rows 32*s+g
            nc.gpsimd.memset(st16[:, :], 1.0)

            LAG = 4
            DEFER = 3
            spb = spool.tile([128, 512], F32)   # four sT regions at 32s offsets
            deferred = None

            for s in range(NSLAB):
                d0 = dpool.tile([128, 512], F32)
                # open the bank's single accumulation group: zero the whole
                # bank so every has_written bit is set exactly once
                nc.tensor.matmul(
                    d0[:, :], ZZ, SEL[:, 0:512],
                    start=True, stop=False, tile_position=(0, 0),
                )

                def emit_sel(item):
                    half, q, z2t = item
                    g0, g1 = 2 * half, 2 * half + 1
                    nc.tensor.matmul(
                        d0[32 * g0 : 32 * g0 + 32, :],
                        SEL[:, 32 * q : 32 * q + 32],
                        z2t[:, 0:512],
                        start=False, stop=False,
                        tile_position=(0, 32 * g0),
                    )
                    nc.tensor.matmul(
                        d0[32 * g1 : 32 * g1 + 32, :],
                        SEL[:, 32 * q : 32 * q + 32],
                        z2t[:, 512:1024],
                        start=False, stop=False,
                        tile_position=(0, 32 * g1),
                    )

                def emit_epilogue(item):
                    ps, pET = item
                    nc.tensor.matmul(
                        spb[32 * ps : 32 * ps + 4, :], ones4[:, :], pET[:, :],
                        start=True, stop=True, tile_position=(0, 32 * ps),
                    )
                    nc.vector.tensor_scalar_add(
                        st16[32 * ps : 32 * ps + 4, :],
                        spb[32 * ps : 32 * ps + 4, :], 0.0,
                    )

                pend = []
                slot = 0
                for half in range(2):
                    c = 2 * s + half
                    for q in range(NQ):
                        zt = zpool.tile([128, 1024], F32)
                        nc.tensor.matmul(
                            zt[:, 0:512],
                            GG[0:64, 128 * q : 128 * (q + 1)],
                            xt2[0:64, 512 * c : 512 * (c + 1)],
                            start=True, stop=True, tile_position=(0, 0),
                        )
                        nc.tensor.matmul(
                            zt[:, 512:1024],
                            GG[64:128, 128 * q : 128 * (q + 1)],
                            xt2[64:128, 512 * c : 512 * (c + 1)],
                            start=True, stop=True, tile_position=(64, 0),
                        )
                        z2 = z2pool.tile([128, 1024], BF16)
                        path = _tile_path(c, q)
                        if path == 'A':
                            nc.scalar.activation(
                                out=z2[:, :], in_=zt[:, :],
                                func=mybir.ActivationFunctionType.Square,
                                bias=bbias[:, q : q + 1], scale=1.0,
                            )
                        else:
                            zs = zspool.tile([128, 1024], BF16)
                            eng = nc.vector if path == 'D' else nc.gpsimd
                            for h in (0, 1):
                                hs = slice(512 * h, 512 * (h + 1))
                                nc.vector.tensor_scalar_add(
                                    zs[:, hs], zt[:, hs], bbias[:, q : q + 1]
                                )
                                eng.tensor_tensor(
                                    out=z2[:, hs], in0=zs[:, hs], in1=zs[:, hs],
                                    op=mybir.AluOpType.mult,
                                )
                        pend.append((half, q, z2))
                        slot += 1
                        if slot == DEFER and deferred is not None:
                            emit_epilogue(deferred)
                            deferred = None
                        if len(pend) > LAG:
                            emit_sel(pend.pop(0))
                for item in pend:
                    emit_sel(item)

                # close the accumulation group (adds zeros, full bank)
                nc.tensor.matmul(
                    d0[:, :], ZZ, SEL[:, 0:512],
                    start=False, stop=True, tile_position=(0, 0),
                )

                # slab epilogue: exp now; group-sum + staging deferred
                ET = etpool.tile([128, 512], BF16)
                nc.scalar.activation(
                    out=ET[:, :], in_=d0[:, :],
                    func=mybir.ActivationFunctionType.Exp,
                    bias=ebias[:, 0:1], scale=1.0,
                )
                deferred = (s, ET)

            emit_epilogue(deferred)

            # ---- final: ln + bias over all 16 groups, one DMA out ----
            ll16 = finpool.tile([128, 512], F32)
            nc.scalar.activation(
                out=ll16[:, :], in_=st16[:, :],
                func=mybir.ActivationFunctionType.Ln,
            )
            llb = finpool.tile([128, 512], F32)
            nc.scalar.activation(
                out=llb[:, :], in_=ll16[:, :],
                func=mybir.ActivationFunctionType.Copy,
                bias=float(-EXPB - threshold_f), scale=1.0,
            )
            # llb rows 32 s + g  ->  out_d row 4 s + g
            for s in range(NSLAB):
                nc.sync.dma_start(
                    out=out_d[4 * s : 4 * s + 4, :],
                    in_=llb[32 * s : 32 * s + 4, :],
                )

    nc.compile()
    return nc


def _host_prep(X, center, cov_inv_sqrt, weight, threshold):
    import ml_dtypes

    BFD = ml_dtypes.bfloat16
    L = cov_inv_sqrt.astype(np.float64)
    w = np.abs(weight.astype(np.float64))
    pr = w / w.sum()
    sign, logdetL = np.linalg.slogdet(L)
    lncoef = np.log(pr) + logdetL                          # [K]
    b = np.einsum("kde,kd->ke", L, center.astype(np.float64))  # b_k = L_k^T c_k

    s05 = np.sqrt(0.5)
    Gflat = (s05 * L).transpose(1, 0, 2).reshape(D, K * D)  # [64, 2048]
    GG = np.concatenate([Gflat, Gflat], axis=0)             # [128, 2048]

    SEL = np.zeros((128, 512), np.float64)
    for q in range(NQ):
        SEL[0:64, 32 * q + 2 * q] = -1.0
        SEL[64:128, 32 * q + 2 * q + 1] = -1.0
    ones4 = np.zeros((128, 4), np.float64)
    for j in range(4):
        ones4[32 * j : 32 * j + 32, j] = 1.0
    ZZ = np.zeros((128, 128), np.float64)

    gsb = np.concatenate([GG, SEL, ones4, ZZ], axis=1).astype(BFD)  # [128, 2692]

    bbias = np.zeros((128, 16), np.float64)
    for q in range(NQ):
        bbias[0:64, q] = -s05 * b[2 * q]
        bbias[64:128, q] = -s05 * b[2 * q + 1]
    ebias = np.tile(lncoef + EXPB, 4)[:, None]                  # [128, 1]
    fb = np.concatenate([bbias, ebias], axis=1).astype(np.float32)

    # XT2 per core: [128, 4096]
    X8 = X.reshape(NCORES, NCH, 2, 512, D)
    XT2 = X8.transpose(0, 2, 4, 1, 3).reshape(NCORES, 128, NCH * 512).astype(BFD)

    thr = float(np.asarray(threshold, dtype=np.float64))
    return XT2, gsb, fb, thr


_CACHE = {}


def kernel(X, center, cov_inv_sqrt, weight, threshold):
    XT2, gsb, fb, thr = _host_prep(X, center, cov_inv_sqrt, weight, threshold)

    key = ("nc", thr)
    if key not in _CACHE:
        _CACHE[key] = _build_nc(thr)
    nc = _CACHE[key]

    in_maps = []
    for i in range(NCORES):
        in_maps.append({
            "xa": np.ascontiguousarray(XT2[i]),
            "gsb": gsb,
            "fb": fb,
        })

    res = run_bass_kernel_spmd(nc, in_maps, core_ids=list(range(NCORES)))
    outs = res.results
    ll = np.concatenate(
        [np.asarray(outs[i]["out"], dtype=np.float32).reshape(NLOC)
         for i in range(NCORES)]
    )
    return ll


# revision 3
# speedup vs baseline: 1.0012x; 1.0012x over previous
"""GaussianMixture log-likelihood kernel for 8 TRN2 NeuronCores, v2.

Math (per point x, cluster k):
  d_ik = -0.5 ||L_k^T x - b_k||^2,  b_k = L_k^T c_k,  L = cov_inv_sqrt
  ll_i = log sum_k coef_k exp(d_ik) - thr,  coef_k = pr_k |det L_k|

Device strategy (data-parallel over N, 8192 points/core), "orientation B":
  - X^T lives in SBUF as [128, 4096] bf16: rows 0:64 = features of the first
    512 points of each 1024-point column-chunk, rows 64:128 = the second 512.
  - Whitening matmuls run as row-packed pairs (tile_position rows 0/64): two
    K=64 matmuls concurrently, producing Z^T chunks [128 = 2 clusters x 64
    coords, 512 points] in PSUM. G is pre-scaled by sqrt(0.5).
  - Squares: ACT activation(Square, bias=-sqrt(.5) b per partition) or DVE
    (tensor_scalar_add bias then tensor_tensor mult) -> 0.5(z-b)^2 bf16.
  - c-reduction on the PE: selector matmuls (SEL = -1 block pattern)
    accumulate -0.5||z-b||^2 into a [128 = 4 groups x 32 clusters, 512] PSUM
    slab; the two per-slot selector mms use different col-groups and run
    concurrently.
  - Epilogue per slab, all ACT: E = exp(d0 + (ln coef + 84)) bf16, s = ones
    matmul over the 32-cluster groups, ll = ln(s) - 84 - thr, DMA out.
"""

import sys

sys.path.insert(0, "/opt/trn_rl_repo")

import numpy as np

from concourse import bacc, mybir
from concourse.tile import TileContext
from concourse.bass_utils import run_bass_kernel_spmd

N, D, K = 65536, 64, 32
NCORES = 8
NLOC = N // NCORES            # 8192 points per core
NCH = 8                       # column chunks of 512 XT2 cols (1024 points)
NQ = 16                       # cluster chunk-pairs (2 clusters x 64 coords)
NSLAB = 4                     # slabs of 2 col-chunks (4 point-groups of 512)
EXPB = 38.0

F32 = mybir.dt.float32
BF16 = mybir.dt.bfloat16

# square-path assignment per tile index mod 16:
#   A = ACT square(bias), D = DVE ts_add + DVE tt mult, P = DVE ts_add + Pool tt
_PATH32 = ['A', 'P', 'A', 'D', 'A', 'P', 'A', 'A',
           'P', 'A', 'A', 'P', 'A', 'A', 'P', 'A',
           'D', 'P', 'A', 'A', 'P', 'A', 'A', 'P',
           'A', 'D', 'A', 'P', 'A', 'A', 'P', 'A']


def _tile_path(c: int, q: int) -> str:
    return _PATH32[(c * NQ + q) % 32]


def _build_nc(threshold_f: float):
    nc = bacc.Bacc()

    xa_d = nc.declare_dram_parameter("xa", [128, 4096], BF16, isOutput=False)
    # gsb: GG [128,2048] | SEL [128,512] | ones4 [128,4] | ZZ [128,128]
    gsb_d = nc.declare_dram_parameter("gsb", [128, 2048 + 512 + 4 + 128], BF16, isOutput=False)
    # fbias: bbias [128,16] | ebias [128,1]
    fb_d = nc.declare_dram_parameter("fb", [128, 17], F32, isOutput=False)
    out_d = nc.declare_dram_parameter("out", [16, 512], F32, isOutput=True)

    with TileContext(nc) as tc:
        with (
            tc.tile_pool(name="const", bufs=1) as cpool,
            tc.tile_pool(name="z2", bufs=9) as z2pool,
            tc.tile_pool(name="zs", bufs=6) as zspool,
            tc.tile_pool(name="et", bufs=2) as etpool,
            tc.tile_pool(name="fin", bufs=2) as finpool,
            tc.tile_pool(name="psz", bufs=3, space="PSUM") as zpool,
            tc.tile_pool(name="psd", bufs=1, space="PSUM") as dpool,
            tc.tile_pool(name="pss", bufs=1, space="PSUM") as spool,
        ):
            xt2 = cpool.tile([128, 4096], BF16)
            nc.sync.dma_start(out=xt2[:, :], in_=xa_d[:, :])
            gsb = cpool.tile([128, 2048 + 512 + 4 + 128], BF16)
            nc.sync.dma_start(out=gsb[:, :], in_=gsb_d[:, :])
            fb = cpool.tile([128, 17], F32)
            nc.sync.dma_start(out=fb[:, :], in_=fb_d[:, :])

            GG = gsb[:, 0:2048]
            SEL = gsb[:, 2048:2560]
            ones4 = gsb[:, 2560:2564]
            ZZ = gsb[:, 2564:2692]
            bbias = fb[:, 0:16]
            ebias = fb[:, 16:17]
            st16 = cpool.tile([128, 512], F32)  # rows 32*s+g
            nc.gpsimd.memset(st16[:, :], 1.0)

            LAG = 6
            DEFER = 3
            spb = spool.tile([128, 512], F32)   # four sT regions at 32s offsets
            deferred = None

            for s in range(NSLAB):
                d0 = dpool.tile([128, 512], F32)
                # open the bank's single accumulation group: zero the whole
                # bank so every has_written bit is set exactly once
                nc.tensor.matmul(
                    d0[:, :], ZZ, SEL[:, 0:512],
                    start=True, stop=False, tile_position=(0, 0),
                )

                def emit_sel(item):
                    half, q, z2t = item
                    g0, g1 = 2 * half, 2 * half + 1
                    nc.tensor.matmul(
                        d0[32 * g0 : 32 * g0 + 32, :],
                        SEL[:, 32 * q : 32 * q + 32],
                        z2t[:, 0:512],
                        start=False, stop=False,
                        tile_position=(0, 32 * g0),
                    )
                    nc.tensor.matmul(
                        d0[32 * g1 : 32 * g1 + 32, :],
                        SEL[:, 32 * q : 32 * q + 32],
                        z2t[:, 512:1024],
                        start=False, stop=False,
                        tile_position=(0, 32 * g1),
                    )

                def emit_epilogue(item):
                    ps, pET = item
                    nc.tensor.matmul(
                        spb[32 * ps : 32 * ps + 4, :], ones4[:, :], pET[:, :],
                        start=True, stop=True, tile_position=(0, 32 * ps),
                    )
                    nc.vector.tensor_scalar_add(
                        st16[32 * ps : 32 * ps + 4, :],
                        spb[32 * ps : 32 * ps + 4, :], 0.0,
                    )

                pend = []
                slot = 0
                for half in range(2):
                    c = 2 * s + half
                    for q in range(NQ):
                        zt = zpool.tile([128, 1024], F32)
                        nc.tensor.matmul(
                            zt[:, 0:512],
                            GG[0:64, 128 * q : 128 * (q + 1)],
                            xt2[0:64, 512 * c : 512 * (c + 1)],
                            start=True, stop=True, tile_position=(0, 0),
                        )
                        nc.tensor.matmul(
                            zt[:, 512:1024],
                            GG[64:128, 128 * q : 128 * (q + 1)],
                            xt2[64:128, 512 * c : 512 * (c + 1)],
                            start=True, stop=True, tile_position=(64, 0),
                        )
                        z2 = z2pool.tile([128, 1024], BF16)
                        path = _tile_path(c, q)
                        if path == 'A':
                            nc.scalar.activation(
                                out=z2[:, :], in_=zt[:, :],
                                func=mybir.ActivationFunctionType.Square,
                                bias=bbias[:, q : q + 1], scale=1.0,
                            )
                        else:
                            zs = zspool.tile([128, 1024], BF16)
                            eng = nc.vector if path == 'D' else nc.gpsimd
                            for h in (0, 1):
                                hs = slice(512 * h, 512 * (h + 1))
                                nc.vector.tensor_scalar_add(
                                    zs[:, hs], zt[:, hs], bbias[:, q : q + 1]
                                )
                                eng.tensor_tensor(
                                    out=z2[:, hs], in0=zs[:, hs], in1=zs[:, hs],
                                    op=mybir.AluOpType.mult,
                                )
                        pend.append((half, q, z2))
                        slot += 1
                        if slot == DEFER and deferred is not None:
                            emit_epilogue(deferred)
                            deferred = None
                        if len(pend) > LAG:
                            emit_sel(pend.pop(0))
                for item in pend:
                    emit_sel(item)

                # close the accumulation group (adds zeros, full bank)
                nc.tensor.matmul(
                    d0[:, :], ZZ, SEL[:, 0:512],
                    start=False, stop=True, tile_position=(0, 0),
                )

                # slab epilogue: exp now; group-sum + staging deferred
                ET = etpool.tile([128, 512], BF16)
                nc.scalar.activation(
                    out=ET[:, :], in_=d0[:, :],
                    func=mybir.ActivationFunctionType.Exp,
                    bias=ebias[:, 0:1], scale=1.0,
                )
                deferred = (s, ET)

            emit_epilogue(deferred)

            # ---- final: ln + bias over all 16 groups, one DMA out ----
            ll16 = finpool.tile([128, 512], F32)
            nc.scalar.activation(
                out=ll16[:, :], in_=st16[:, :],
                func=mybir.ActivationFunctionType.Ln,
            )
            llb = finpool.tile([128, 512], F32)
            nc.scalar.activation(
                out=llb[:, :], in_=ll16[:, :],
                func=mybir.ActivationFunctionType.Copy,
                bias=float(-EXPB - threshold_f), scale=1.0,
            )
            # llb rows 32 s + g  ->  out_d row 4 s + g
            for s in range(NSLAB):
                nc.sync.dma_start(
                    out=out_d[4 * s : 4 * s + 4, :],
                    in_=llb[32 * s : 32 * s + 4, :],
                )

    nc.compile()
    return nc


def _host_prep(X, center, cov_inv_sqrt, weight, threshold):
    import ml_dtypes

    BFD = ml_dtypes.bfloat16
    L = cov_inv_sqrt.astype(np.float64)
    w = np.abs(weight.astype(np.float64))
    pr = w / w.sum()
    sign, logdetL = np.linalg.slogdet(L)
    lncoef = np.log(pr) + logdetL                          # [K]
    b = np.einsum("kde,kd->ke", L, center.astype(np.float64))  # b_k = L_k^T c_k

    s05 = np.sqrt(0.5)
    Gflat = (s05 * L).transpose(1, 0, 2).reshape(D, K * D)  # [64, 2048]
    GG = np.concatenate([Gflat, Gflat], axis=0)             # [128, 2048]

    SEL = np.zeros((128, 512), np.float64)
    for q in range(NQ):
        SEL[0:64, 32 * q + 2 * q] = -1.0
        SEL[64:128, 32 * q + 2 * q + 1] = -1.0
    ones4 = np.zeros((128, 4), np.float64)
    for j in range(4):
        ones4[32 * j : 32 * j + 32, j] = 1.0
    ZZ = np.zeros((128, 128), np.float64)

    gsb = np.concatenate([GG, SEL, ones4, ZZ], axis=1).astype(BFD)  # [128, 2692]

    bbias = np.zeros((128, 16), np.float64)
    for q in range(NQ):
        bbias[0:64, q] = -s05 * b[2 * q]
        bbias[64:128, q] = -s05 * b[2 * q + 1]
    ebias = np.tile(lncoef + EXPB, 4)[:, None]                  # [128, 1]
    fb = np.concatenate([bbias, ebias], axis=1).astype(np.float32)

    # XT2 per core: [128, 4096]
    X8 = X.reshape(NCORES, NCH, 2, 512, D)
    XT2 = X8.transpose(0, 2, 4, 1, 3).reshape(NCORES, 128, NCH * 512).astype(BFD)

    thr = float(np.asarray(threshold, dtype=np.float64))
    return XT2, gsb, fb, thr


_CACHE = {}


def kernel(X, center, cov_inv_sqrt, weight, threshold):
    XT2, gsb, fb, thr = _host_prep(X, center, cov_inv_sqrt, weight, threshold)

    key = ("nc", thr)
    if key not in _CACHE:
        _CACHE[key] = _build_nc(thr)
    nc = _CACHE[key]

    in_maps = []
    for i in range(NCORES):
        in_maps.append({
            "xa": np.ascontiguousarray(XT2[i]),
            "gsb": gsb,
            "fb": fb,
        })

    res = run_bass_kernel_spmd(nc, in_maps, core_ids=list(range(NCORES)))
    outs = res.results
    ll = np.concatenate(
        [np.asarray(outs[i]["out"], dtype=np.float32).reshape(NLOC)
         for i in range(NCORES)]
    )
    return ll
